# revision 9
# baseline (speedup 1.0000x reference)
"""Differential attention kernel for 8 Trainium2 NeuronCores — v4.

Sharding: (batch, key-half) per core. Each core projects K/V for its
2048-key half only (dedup vs v3's query-split, which duplicated K/V
across the pair), projects Q for all 4096 queries, and computes partial
PV + row-sum accumulators for both heads over its key half. The host
sums the two partials per batch and applies the differential-softmax
normalization (o1/r1 - lam*o2/r2) in numpy.

Projections run as fp8-e4m3 DoubleRow matmuls (cost 0.5 cycles/row,
256-wide contraction) with a 3-term hi/lo residual split:
  x @ W  ~=  xh@Wh + xl@Wh + xh@Wl      (lo*lo term dropped, ~eps^2)
which keeps bf16-level accuracy (measured 3.9e-3 vs 4.5e-3 all-bf16)
at 0.75x the PE cycles. Scores and PV stay bf16 (fp8 cannot hold the
exp() dynamic range; measured catastrophic).

Host preps x/W hi+lo splits and interleaved DRAM layouts so each weight
is one DMA and each x chunk is two (hi/lo).
"""

import math
import os
import time
from contextlib import ExitStack

import ml_dtypes
import numpy as np

import concourse.bass as bass
from concourse import bacc
import concourse.mybir as mybir
import concourse.tile as tile
from concourse.bass_utils import run_bass_kernel_spmd

B, S, D = 4, 4096, 2048
HD = 128
DV = 256
DVA = DV + 1      # + ones column for row sums
SK = S // 2       # keys per core (key-half)
N_CORES = 8
DEPTH = 12
SCALE = HD ** -0.5
WSC = 64.0        # host-side weight scale before fp8 split

DT_P = D // 128   # 16 d-tiles
DP = DT_P // 2    # 8 d-pairs (DoubleRow contraction = 256)
SC = S // 512     # 8 s-chunks (queries)
KC = SK // 512    # 4 s-chunks that are also key chunks
SKT = SK // 128   # 16 key tiles
QC = S // 512     # 8 attention q-chunks
QT = S // 128     # 32 q tiles

BF16 = mybir.dt.bfloat16
F32 = mybir.dt.float32
FP8 = mybir.dt.float8e4
DR = mybir.MatmulPerfMode.DoubleRow
E4 = ml_dtypes.float8_e4m3

INPUT_NAMES = ("xh", "xl", "wqh", "wql", "wkh", "wkl", "wvh", "wvl")

_cache = {}


def build_nc():
    nc = bacc.Bacc("TRN2", target_bir_lowering=False, debug=False)

    # x split halves, host-arranged as [p, dpair, j, chunk, col] so one
    # (chunk) slice is a 2-descriptor-per-partition DMA.
    xh_d = nc.declare_dram_parameter("xh", [128, DP, 2, SC, 512], FP8, isOutput=False)
    xl_d = nc.declare_dram_parameter("xl", [128, DP, 2, SC, 512], FP8, isOutput=False)
    w_d = {}
    for n in ("wqh", "wql", "wkh", "wkl", "wvh", "wvl"):
        # host-arranged [p, dtile, col]
        w_d[n] = nc.declare_dram_parameter(n, [128, DT_P, DV], FP8, isOutput=False)
    # out[h, p, qtile, dva]: per-(qc, h) DMA is an exact [128, 4, 257] match
    out_d = nc.declare_dram_parameter("out", [2, 128, QT, DVA], F32, isOutput=True)

    out = out_d.ap()

    with tile.TileContext(nc) as tc, ExitStack() as ctx:
        singles = ctx.enter_context(tc.tile_pool(name="singles", bufs=1))
        x_pool = ctx.enter_context(tc.tile_pool(name="x", bufs=6))
        e_pool = ctx.enter_context(tc.tile_pool(name="e", bufs=6))
        o_pool = ctx.enter_context(tc.tile_pool(name="o", bufs=4))

        # --- resident SBUF tensors --------------------------------------
        w_sb = {n: singles.tile([128, DT_P, DV], FP8, tag=f"w_{n}", name=f"w_{n}")
                for n in w_d}

        x_tiles = {}

        def dma_x(sc):
            xh_t = x_pool.tile([128, DP, 2, 512], FP8, tag="xt", name=f"xh{sc}")
            xl_t = x_pool.tile([128, DP, 2, 512], FP8, tag="xt", name=f"xl{sc}")
            nc.sync.dma_start(out=xh_t, in_=xh_d.ap()[:, :, :, sc, :])
            nc.sync.dma_start(out=xl_t, in_=xl_d.ap()[:, :, :, sc, :])
            x_tiles[sc] = (xh_t, xl_t)

        kT = singles.tile([128, 2, SK], BF16, tag="kT")       # [dh, head, sk]
        qT = singles.tile([128, 2, S], BF16, tag="qT")        # [dh, head, sq]
        v_aug = singles.tile([128, SKT, DVA], BF16, tag="v")  # [s_row, s_tile, dv+1]

        # ones column carries the row sums; 64 cancels the weight scale
        nc.vector.memset(v_aug[:, :, DV:DVA], WSC)

        # DMA issue order: what the first chunk's matmuls need comes first
        # (K terms need wkh+xh0, then xl0/wkl), rest follows.
        nc.sync.dma_start(out=w_sb["wkh"], in_=w_d["wkh"].ap())
        dma_x(0)
        for n in ("wkl", "wqh", "wql", "wvh", "wvl"):
            nc.sync.dma_start(out=w_sb[n], in_=w_d[n].ap())
        dma_x(1)

        # --- projections: one pass over the 8 s-chunks ------------------
        pctx = ExitStack()
        psum = pctx.enter_context(
            tc.tile_pool(name="psum_proj", bufs=4, space=bass.MemorySpace.PSUM)
        )

        # PE warm-up: junk matmuls during the initial DMA wait so the PE
        # p-state is ramped when the first real matmul issues.
        jt = singles.tile([128, 512], BF16, tag="junk")
        nc.vector.memset(jt, 0.0)
        jps = psum.tile([128, 512], F32, tag="big_ps", bufs=4, name="jps")
        for w in range(12):
            nc.tensor.matmul(jps, jt[:, 0:128], jt, start=True, stop=True)
        nc.vector.tensor_copy(jt, jps)

        def proj_cols(ps, wname, hsl, xh_t, xl_t, col0, ncol):
            """3-term DoubleRow accumulation of one [128, ncol] output."""
            first = True
            for wn, xt in ((wname + "h", xh_t), (wname + "h", xl_t),
                           (wname + "l", xh_t)):
                for dp in range(DP):
                    nc.tensor.matmul(
                        ps,
                        w_sb[wn][:, 2 * dp:2 * dp + 2, hsl],
                        xt[:, dp, :, col0:col0 + ncol],
                        start=first,
                        stop=(wn == wname + "l" and dp == DP - 1),
                        perf_mode=DR,
                    )
                    first = False

        for sc in range(SC):
            if sc + 2 < SC:
                dma_x(sc + 2)
            xh_t, xl_t = x_tiles.pop(sc)

            projs = ([("wk", kT)] if sc < KC else []) + [("wq", qT)]
            for wname, dst in projs:
                for h in range(2):
                    ps = psum.tile([128, 512], F32, tag="big_ps", bufs=4,
                                   name=f"ps{sc}{wname}{h}")
                    proj_cols(ps, wname, slice(h * HD, (h + 1) * HD),
                              xh_t, xl_t, 0, 512)
                    nc.vector.tensor_copy(dst[:, h, sc * 512:(sc + 1) * 512], ps)
            if sc < KC:
                for i in range(4):
                    vps = psum.tile([128, DV], F32, tag="v_ps", bufs=2,
                                    name=f"vps{sc}_{i}")
                    first = True
                    for wn, xt in (("wvh", xh_t), ("wvh", xl_t), ("wvl", xh_t)):
                        for dp in range(DP):
                            nc.tensor.matmul(
                                vps,
                                xt[:, dp, :, i * 128:(i + 1) * 128],
                                w_sb[wn][:, 2 * dp:2 * dp + 2, :],
                                start=first,
                                stop=(wn == "wvl" and dp == DP - 1),
                                perf_mode=DR,
                            )
                            first = False
                    nc.vector.tensor_copy(v_aug[:, sc * 4 + i, 0:DV], vps)

        pctx.close()

        # --- attention: per (head, q-chunk), partial PV over key half ---
        psum = ctx.enter_context(
            tc.tile_pool(name="psum_att", bufs=4, space=bass.MemorySpace.PSUM)
        )
        psum_pv = ctx.enter_context(
            tc.tile_pool(name="psum_pv", bufs=4, space=bass.MemorySpace.PSUM)
        )
        for h in range(2):
            for qc in range(QC):
                pv_ps = [
                    psum_pv.tile([128, DVA], F32, tag="pv_ps", name=f"pv_ps{i}")
                    for i in range(4)
                ]
                for skp in range(SKT // 2):
                    # two key tiles' scores land in one 2-bank psum tile so a
                    # single exp instruction covers both (halves ACT count)
                    sps = psum.tile([128, 2, 512], F32, tag="sps", bufs=2,
                                    name=f"sps{skp}")
                    for j in range(2):
                        nc.tensor.matmul(
                            sps[:, j, :],
                            kT[:, h, (2 * skp + j) * 128:(2 * skp + j + 1) * 128],
                            qT[:, h, qc * 512:(qc + 1) * 512],
                        )
                    et = e_pool.tile([128, 2, 512], BF16, tag="et", name=f"et{skp}")
                    nc.scalar.activation(
                        out=et, in_=sps,
                        func=mybir.ActivationFunctionType.Exp,
                        scale=SCALE / (WSC * WSC),
                    )
                    for j in range(2):
                        skt = 2 * skp + j
                        for i in range(4):
                            nc.tensor.matmul(
                                pv_ps[i],
                                et[:, j, i * 128:(i + 1) * 128],
                                v_aug[:, skt, :],
                                start=(skt == 0),
                                stop=(skt == SKT - 1),
                            )
                ot = o_pool.tile([128, 4, DVA], F32, tag="ot", name=f"ot{h}{qc}")
                for i in range(4):
                    nc.vector.tensor_copy(ot[:, i, :], pv_ps[i])
                nc.sync.dma_start(
                    out=out[h, :, qc * 4:(qc + 1) * 4, :], in_=ot
                )

    nc.compile()
    return nc


def _lam(lambda_q1, lambda_q2, lambda_k1, lambda_k2):
    lam_init = 0.8 - 0.6 * math.exp(-0.3 * DEPTH)
    l1 = math.exp(float(np.sum(lambda_q1.astype(np.float64) * lambda_k1.astype(np.float64))))
    l2 = math.exp(float(np.sum(lambda_q2.astype(np.float64) * lambda_k2.astype(np.float64))))
    return l1 + l2 + lam_init


def _split_x(xT):
    """xT [D, S] f32 -> (hi, lo) e4m3 in [128, DP, 2, SC, 512] layout."""
    xh = xT.astype(E4)
    xl = (xT - xh.astype(np.float32)).astype(E4)
    out = []
    for a in (xh, xl):
        a = a.reshape(DP, 2, 128, SC, 512).transpose(2, 0, 1, 3, 4)
        out.append(np.ascontiguousarray(a))
    return out


def _split_w(W):
    """W [D, DV] f32 -> (hi, lo) e4m3 in [128, DT_P, DV] layout."""
    Ws = W.astype(np.float32) * WSC
    wh = Ws.astype(E4)
    wl = (Ws - wh.astype(np.float32)).astype(E4)
    out = []
    for a in (wh, wl):
        a = a.reshape(DT_P, 128, DV).transpose(1, 0, 2)
        out.append(np.ascontiguousarray(a))
    return out


def kernel(x, WQ, WK, WV, lambda_q1, lambda_q2, lambda_k1, lambda_k2):
    if "nc" not in _cache:
        _cache["nc"] = build_nc()
    nc = _cache["nc"]

    wqh, wql = _split_w(WQ)
    wkh, wkl = _split_w(WK)
    wvh, wvl = _split_w(WV)
    lam = _lam(lambda_q1, lambda_q2, lambda_k1, lambda_k2)

    in_maps = []
    for c in range(N_CORES):
        b, kh = c // 2, c % 2
        xb = x[b] if kh == 0 else np.concatenate([x[b, SK:], x[b, :SK]], axis=0)
        xT = np.ascontiguousarray(xb.T, dtype=np.float32)
        xh, xl = _split_x(xT)
        in_maps.append({
            "xh": xh, "xl": xl,
            "wqh": wqh, "wql": wql, "wkh": wkh, "wkl": wkl,
            "wvh": wvh, "wvl": wvl,
        })

    kres = None
    for attempt in range(3):
        try:
            kres = run_bass_kernel_spmd(nc, in_maps, list(range(N_CORES)))
            break
        except (ModuleNotFoundError, ImportError):
            # BASS_TRACE requested but this axon build has no NTFF hook
            os.environ["BASS_NEVER_TRACE"] = "1"
        except Exception:
            if attempt == 2:
                raise
            time.sleep(5)
    if kres is None:
        kres = run_bass_kernel_spmd(nc, in_maps, list(range(N_CORES)))
    _cache["last_results"] = kres
    res = kres.results

    out = np.empty((B, S, DV), np.float32)
    for b in range(B):
        # out tensor is [2, 128, QT, DVA]: query index = qt*128 + p
        a0 = res[2 * b]["out"].transpose(0, 2, 1, 3).reshape(2, S, DVA)
        a1 = res[2 * b + 1]["out"].transpose(0, 2, 1, 3).reshape(2, S, DVA)
        a1 = np.concatenate([a1[:, SK:], a1[:, :SK]], axis=1)  # un-rotate
        acc = a0.astype(np.float64) + a1.astype(np.float64)
        o1 = acc[0, :, :DV] / acc[0, :, DV:DVA]
        o2 = acc[1, :, :DV] / acc[1, :, DV:DVA]
        out[b] = (o1 - lam * o2).astype(np.float32)
    return out


# revision 17
# speedup vs baseline: 1.1911x; 1.1911x over previous
"""Differential attention kernel for 8 Trainium2 NeuronCores — v4.

Sharding: (batch, key-half) per core. Each core projects K/V for its
2048-key half only (dedup vs v3's query-split, which duplicated K/V
across the pair), projects Q for all 4096 queries, and computes partial
PV + row-sum accumulators for both heads over its key half. The host
sums the two partials per batch and applies the differential-softmax
normalization (o1/r1 - lam*o2/r2) in numpy.

Projections run as fp8-e4m3 DoubleRow matmuls (cost 0.5 cycles/row,
256-wide contraction) with a 3-term hi/lo residual split:
  x @ W  ~=  xh@Wh + xl@Wh + xh@Wl      (lo*lo term dropped, ~eps^2)
which keeps bf16-level accuracy (measured 3.9e-3 vs 4.5e-3 all-bf16)
at 0.75x the PE cycles. Scores and PV stay bf16 (fp8 cannot hold the
exp() dynamic range; measured catastrophic).

Host preps x/W hi+lo splits and interleaved DRAM layouts so each weight
is one DMA and each x chunk is two (hi/lo).
"""

import math
import os
import time
from contextlib import ExitStack

import ml_dtypes
import numpy as np

import concourse.bass as bass
from concourse import bacc
import concourse.mybir as mybir
import concourse.tile as tile
from concourse.bass_utils import run_bass_kernel_spmd

B, S, D = 4, 4096, 2048
HD = 128
DV = 256
DVA = DV + 1      # + ones column for row sums
SK = S // 2       # keys per core (key-half)
N_CORES = 8
DEPTH = 12
SCALE = HD ** -0.5
WSC = 64.0        # host-side weight scale before fp8 split

DT_P = D // 128   # 16 d-tiles
DP = DT_P // 2    # 8 d-pairs (DoubleRow contraction = 256)
SC = S // 512     # 8 s-chunks (queries)
KC = SK // 512    # 4 s-chunks that are also key chunks
SKT = SK // 128   # 16 key tiles
QC = S // 512     # 8 attention q-chunks
QT = S // 128     # 32 q tiles

BF16 = mybir.dt.bfloat16
F32 = mybir.dt.float32
FP8 = mybir.dt.float8e4
DR = mybir.MatmulPerfMode.DoubleRow
E4 = ml_dtypes.float8_e4m3

INPUT_NAMES = ("xh", "xl", "wqh", "wql", "wkh", "wkl", "wvh", "wvl")

_cache = {}


def build_nc():
    nc = bacc.Bacc("TRN2", target_bir_lowering=False, debug=False)

    # x split halves, host-arranged as [p, chunk, dpair, j, col] so one
    # chunk slice is a single contiguous 8KB descriptor per partition.
    xh_d = nc.declare_dram_parameter("xh", [128, SC, DP, 2, 512], FP8, isOutput=False)
    xl_d = nc.declare_dram_parameter("xl", [128, SC, DP, 2, 512], FP8, isOutput=False)
    w_d = {}
    for n in ("wqh", "wql", "wkh", "wkl", "wvh", "wvl"):
        # host-arranged [p, dtile, col]
        w_d[n] = nc.declare_dram_parameter(n, [128, DT_P, DV], FP8, isOutput=False)
    # out[p, qc, h, sub, dva]: one contiguous-per-partition DMA per q-chunk
    out_d = nc.declare_dram_parameter("out", [128, QC, 2, 4, DVA], F32, isOutput=True)

    out = out_d.ap()

    with tile.TileContext(nc) as tc, ExitStack() as ctx:
        singles = ctx.enter_context(tc.tile_pool(name="singles", bufs=1))
        x_pool = ctx.enter_context(tc.tile_pool(name="x", bufs=6))
        e_pool = ctx.enter_context(tc.tile_pool(name="e", bufs=6))
        o_pool = ctx.enter_context(tc.tile_pool(name="o", bufs=4))

        # --- resident SBUF tensors --------------------------------------
        w_sb = {n: singles.tile([128, DT_P, DV], FP8, tag=f"w_{n}", name=f"w_{n}")
                for n in w_d}

        x_tiles = {}

        def dma_x(sc):
            xh_t = x_pool.tile([128, DP, 2, 512], FP8, tag="xt", name=f"xh{sc}")
            xl_t = x_pool.tile([128, DP, 2, 512], FP8, tag="xt", name=f"xl{sc}")
            nc.sync.dma_start(out=xh_t, in_=xh_d.ap()[:, sc, :, :, :])
            nc.sync.dma_start(out=xl_t, in_=xl_d.ap()[:, sc, :, :, :])
            x_tiles[sc] = (xh_t, xl_t)

        kT = singles.tile([128, 2, SK], BF16, tag="kT")       # [dh, head, sk]
        qT = singles.tile([128, 2, S], BF16, tag="qT")        # [dh, head, sq]
        v_aug = singles.tile([128, SKT, DVA], BF16, tag="v")  # [s_row, s_tile, dv+1]

        # ones column carries the row sums; 64 cancels the weight scale
        nc.vector.memset(v_aug[:, :, DV:DVA], WSC)

        # DMA issue order: what the first chunk's matmuls need comes first
        # (K terms need wkh+xh0, then xl0/wkl), rest follows.
        nc.sync.dma_start(out=w_sb["wkh"], in_=w_d["wkh"].ap())
        dma_x(0)
        for n in ("wkl", "wqh", "wql", "wvh", "wvl"):
            nc.sync.dma_start(out=w_sb[n], in_=w_d[n].ap())
        dma_x(1)

        # --- projections: one pass over the 8 s-chunks ------------------
        pctx = ExitStack()
        psum = pctx.enter_context(
            tc.tile_pool(name="psum_proj", bufs=4, space=bass.MemorySpace.PSUM)
        )

        # PE warm-up: junk matmuls during the initial DMA wait so the PE
        # p-state is ramped when the first real matmul issues.
        # tiny-FD matmuls: enough wall-clock to ramp the p-state and cover
        # the first DMAs, at minimal cycle cost
        jt = singles.tile([128, 512], BF16, tag="junk")
        nc.vector.memset(jt, 0.0)
        jps = psum.tile([128, 512], F32, tag="big_ps", bufs=4, name="jps")
        for w in range(48):
            nc.tensor.matmul(jps[:, 0:64], jt[:, 0:128], jt[:, 0:64],
                             start=True, stop=True)
        nc.vector.tensor_copy(jt[:, 0:64], jps[:, 0:64])

        def proj_cols(ps, wname, hsl, xh_t, xl_t, col0, ncol):
            """3-term DoubleRow accumulation of one [128, ncol] output."""
            first = True
            for wn, xt in ((wname + "h", xh_t), (wname + "h", xl_t),
                           (wname + "l", xh_t)):
                for dp in range(DP):
                    nc.tensor.matmul(
                        ps,
                        w_sb[wn][:, 2 * dp:2 * dp + 2, hsl],
                        xt[:, dp, :, col0:col0 + ncol],
                        start=first,
                        stop=(wn == wname + "l" and dp == DP - 1),
                        perf_mode=DR,
                    )
                    first = False

        for sc in range(SC):
            if sc + 2 < SC:
                dma_x(sc + 2)
            xh_t, xl_t = x_tiles.pop(sc)

            projs = ([("wk", kT)] if sc < KC else []) + [("wq", qT)]
            for wname, dst in projs:
                for h in range(2):
                    ps = psum.tile([128, 512], F32, tag="big_ps", bufs=4,
                                   name=f"ps{sc}{wname}{h}")
                    proj_cols(ps, wname, slice(h * HD, (h + 1) * HD),
                              xh_t, xl_t, 0, 512)
                    nc.vector.tensor_copy(dst[:, h, sc * 512:(sc + 1) * 512], ps)
            if sc < KC:
                for i in range(4):
                    vps = psum.tile([128, DV], F32, tag="v_ps", bufs=2,
                                    name=f"vps{sc}_{i}")
                    first = True
                    for wn, xt in (("wvh", xh_t), ("wvh", xl_t), ("wvl", xh_t)):
                        for dp in range(DP):
                            nc.tensor.matmul(
                                vps,
                                xt[:, dp, :, i * 128:(i + 1) * 128],
                                w_sb[wn][:, 2 * dp:2 * dp + 2, :],
                                start=first,
                                stop=(wn == "wvl" and dp == DP - 1),
                                perf_mode=DR,
                            )
                            first = False
                    nc.vector.tensor_copy(v_aug[:, sc * 4 + i, 0:DV], vps)

        pctx.close()

        # --- attention: per (head, q-chunk), partial PV over key half ---
        psum = ctx.enter_context(
            tc.tile_pool(name="psum_att", bufs=4, space=bass.MemorySpace.PSUM)
        )
        psum_pv = ctx.enter_context(
            tc.tile_pool(name="psum_pv", bufs=4, space=bass.MemorySpace.PSUM)
        )
        for qc in range(QC):
            ot = o_pool.tile([128, 2, 4, DVA], F32, tag="ot", name=f"ot{qc}")
            for h in range(2):
                pv_ps = [
                    psum_pv.tile([128, DVA], F32, tag="pv_ps", name=f"pv_ps{i}")
                    for i in range(4)
                ]
                for skt in range(SKT):
                    sps = psum.tile([128, 512], F32, tag="sps", bufs=4,
                                    name=f"sps{skt}")
                    nc.tensor.matmul(
                        sps,
                        kT[:, h, skt * 128:(skt + 1) * 128],
                        qT[:, h, qc * 512:(qc + 1) * 512],
                    )
                    et = e_pool.tile([128, 512], BF16, tag="et", name=f"et{skt}")
                    nc.scalar.activation(
                        out=et, in_=sps,
                        func=mybir.ActivationFunctionType.Exp,
                        scale=SCALE / (WSC * WSC),
                    )
                    for i in range(4):
                        nc.tensor.matmul(
                            pv_ps[i],
                            et[:, i * 128:(i + 1) * 128],
                            v_aug[:, skt, :],
                            start=(skt == 0),
                            stop=(skt == SKT - 1),
                        )
                for i in range(4):
                    nc.vector.tensor_copy(ot[:, h, i, :], pv_ps[i])
            nc.sync.dma_start(out=out[:, qc, :, :, :], in_=ot)

    nc.compile()
    return nc


def _lam(lambda_q1, lambda_q2, lambda_k1, lambda_k2):
    lam_init = 0.8 - 0.6 * math.exp(-0.3 * DEPTH)
    l1 = math.exp(float(np.sum(lambda_q1.astype(np.float64) * lambda_k1.astype(np.float64))))
    l2 = math.exp(float(np.sum(lambda_q2.astype(np.float64) * lambda_k2.astype(np.float64))))
    return l1 + l2 + lam_init


def _split_x(xT):
    """xT [D, S] f32 -> (hi, lo) e4m3 in [128, SC, DP, 2, 512] layout."""
    xh = xT.astype(E4)
    xl = (xT - xh.astype(np.float32)).astype(E4)
    out = []
    for a in (xh, xl):
        a = a.reshape(DP, 2, 128, SC, 512).transpose(2, 3, 0, 1, 4)
        out.append(np.ascontiguousarray(a))
    return out


def _split_w(W):
    """W [D, DV] f32 -> (hi, lo) e4m3 in [128, DT_P, DV] layout."""
    Ws = W.astype(np.float32) * WSC
    wh = Ws.astype(E4)
    wl = (Ws - wh.astype(np.float32)).astype(E4)
    out = []
    for a in (wh, wl):
        a = a.reshape(DT_P, 128, DV).transpose(1, 0, 2)
        out.append(np.ascontiguousarray(a))
    return out


def kernel(x, WQ, WK, WV, lambda_q1, lambda_q2, lambda_k1, lambda_k2):
    if "nc" not in _cache:
        _cache["nc"] = build_nc()
    nc = _cache["nc"]

    wqh, wql = _split_w(WQ)
    wkh, wkl = _split_w(WK)
    wvh, wvl = _split_w(WV)
    lam = _lam(lambda_q1, lambda_q2, lambda_k1, lambda_k2)

    in_maps = []
    for c in range(N_CORES):
        b, kh = c // 2, c % 2
        xb = x[b] if kh == 0 else np.concatenate([x[b, SK:], x[b, :SK]], axis=0)
        xT = np.ascontiguousarray(xb.T, dtype=np.float32)
        xh, xl = _split_x(xT)
        in_maps.append({
            "xh": xh, "xl": xl,
            "wqh": wqh, "wql": wql, "wkh": wkh, "wkl": wkl,
            "wvh": wvh, "wvl": wvl,
        })

    kres = None
    for attempt in range(3):
        try:
            kres = run_bass_kernel_spmd(nc, in_maps, list(range(N_CORES)))
            break
        except (ModuleNotFoundError, ImportError):
            # BASS_TRACE requested but this axon build has no NTFF hook
            os.environ["BASS_NEVER_TRACE"] = "1"
        except Exception:
            if attempt == 2:
                raise
            time.sleep(5)
    if kres is None:
        kres = run_bass_kernel_spmd(nc, in_maps, list(range(N_CORES)))
    _cache["last_results"] = kres
    res = kres.results

    out = np.empty((B, S, DV), np.float32)
    for b in range(B):
        # out tensor is [128, QC, 2, 4, DVA]: query index = (qc*4 + sub)*128 + p
        def decode(a):
            # [p, qc, h, sub, dva] -> [h, qc, sub, p, dva] -> [h, S, DVA]
            return a.transpose(2, 1, 3, 0, 4).reshape(2, S, DVA)
        a0 = decode(res[2 * b]["out"])
        a1 = decode(res[2 * b + 1]["out"])
        a1 = np.concatenate([a1[:, SK:], a1[:, :SK]], axis=1)  # un-rotate
        acc = a0.astype(np.float64) + a1.astype(np.float64)
        o1 = acc[0, :, :DV] / acc[0, :, DV:DVA]
        o2 = acc[1, :, :DV] / acc[1, :, DV:DVA]
        out[b] = (o1 - lam * o2).astype(np.float32)
    return out


# revision 37
# speedup vs baseline: 1.2249x; 1.0284x over previous
"""Differential attention kernel for 8 Trainium2 NeuronCores — v4.

Sharding: (batch, key-half) per core. Each core projects K/V for its
2048-key half only (dedup vs v3's query-split, which duplicated K/V
across the pair), projects Q for all 4096 queries, and computes partial
PV + row-sum accumulators for both heads over its key half. The host
sums the two partials per batch and applies the differential-softmax
normalization (o1/r1 - lam*o2/r2) in numpy.

Projections run as fp8-e4m3 DoubleRow matmuls (cost 0.5 cycles/row,
256-wide contraction) with a 3-term hi/lo residual split:
  x @ W  ~=  xh@Wh + xl@Wh + xh@Wl      (lo*lo term dropped, ~eps^2)
which keeps bf16-level accuracy (measured 3.9e-3 vs 4.5e-3 all-bf16)
at 0.75x the PE cycles. Scores and PV stay bf16 (fp8 cannot hold the
exp() dynamic range; measured catastrophic).

Host preps x/W hi+lo splits and interleaved DRAM layouts so each weight
is one DMA and each x chunk is two (hi/lo).
"""

import math
import os
import time
from contextlib import ExitStack

import ml_dtypes
import numpy as np

import concourse.bass as bass
from concourse import bacc
import concourse.mybir as mybir
import concourse.tile as tile
from concourse.bass_utils import run_bass_kernel_spmd

B, S, D = 4, 4096, 2048
HD = 128
DV = 256
DVA = DV + 1      # + ones column for row sums
SK = S // 2       # keys per core (key-half)
N_CORES = 8
DEPTH = 12
SCALE = HD ** -0.5
WSC = 64.0        # host-side weight scale before fp8 split

DT_P = D // 128   # 16 d-tiles
DP = DT_P // 2    # 8 d-pairs (DoubleRow contraction = 256)
SC = S // 512     # 8 s-chunks (queries)
KC = SK // 512    # 4 s-chunks that are also key chunks
SKT = SK // 128   # 16 key tiles
QC = S // 512     # 8 attention q-chunks
QT = S // 128     # 32 q tiles

BF16 = mybir.dt.bfloat16
F32 = mybir.dt.float32
FP8 = mybir.dt.float8e4
DR = mybir.MatmulPerfMode.DoubleRow
E4 = ml_dtypes.float8_e4m3

INPUT_NAMES = ("xh", "xl", "wqh", "wql", "wkh", "wkl", "wvh", "wvl")

_cache = {}


def build_nc():
    nc = bacc.Bacc("TRN2", target_bir_lowering=False, debug=False)

    # x split halves, host-arranged as [p, chunk, dpair, j, col] so one
    # chunk slice is a single contiguous 8KB descriptor per partition.
    xh_d = nc.declare_dram_parameter("xh", [128, SC, DP, 2, 512], FP8, isOutput=False)
    xl_d = nc.declare_dram_parameter("xl", [128, SC, DP, 2, 512], FP8, isOutput=False)
    w_d = {}
    for n in ("wqh", "wql", "wkh", "wkl", "wvh", "wvl"):
        # host-arranged [p, dtile, col]
        w_d[n] = nc.declare_dram_parameter(n, [128, DT_P, DV], FP8, isOutput=False)
    # out[p, qc, h, sub, dva]: one contiguous-per-partition DMA per q-chunk
    out_d = nc.declare_dram_parameter("out", [128, QC, 2, 4, DVA], F32, isOutput=True)

    out = out_d.ap()

    with tile.TileContext(nc) as tc, ExitStack() as ctx:
        singles = ctx.enter_context(tc.tile_pool(name="singles", bufs=1))
        x_pool = ctx.enter_context(tc.tile_pool(name="x", bufs=6))
        e_pool = ctx.enter_context(tc.tile_pool(name="e", bufs=12))
        o_pool = ctx.enter_context(tc.tile_pool(name="o", bufs=4))

        # --- resident SBUF tensors --------------------------------------
        w_sb = {n: singles.tile([128, DT_P, DV], FP8, tag=f"w_{n}", name=f"w_{n}")
                for n in w_d}

        x_tiles = {}

        def dma_x(sc, split=False):
            # DMA issue is split across the otherwise-idle engines: the cost
            # model charges the transfer to the issuing engine's timeline, so
            # xh rides SP while xl rides Pool (concurrent streams).
            xh_t = x_pool.tile([128, DP, 2, 512], FP8, tag="xt", name=f"xh{sc}")
            xl_t = x_pool.tile([128, DP, 2, 512], FP8, tag="xt", name=f"xl{sc}")
            if split:
                # chunk 0: per-dpair pieces so the first matmuls start early
                for dp in range(DP):
                    nc.sync.dma_start(out=xh_t[:, dp, :, :],
                                      in_=xh_d.ap()[:, sc, dp, :, :])
                    nc.gpsimd.dma_start(out=xl_t[:, dp, :, :],
                                        in_=xl_d.ap()[:, sc, dp, :, :])
            else:
                nc.sync.dma_start(out=xh_t, in_=xh_d.ap()[:, sc, :, :, :])
                nc.gpsimd.dma_start(out=xl_t, in_=xl_d.ap()[:, sc, :, :, :])
            x_tiles[sc] = (xh_t, xl_t)

        kT = singles.tile([128, 2, SK], BF16, tag="kT")       # [dh, head, sk]
        qT = singles.tile([128, 2, S], BF16, tag="qT")        # [dh, head, sq]
        v_aug = singles.tile([128, SKT, DVA], BF16, tag="v")  # [s_row, s_tile, dv+1]

        # ones column carries the row sums; 64 cancels the weight scale
        nc.vector.memset(v_aug[:, :, DV:DVA], WSC)

        # DMA issue order: chunk-0 xh pieces stream on SP from t=0 while the
        # first weight rides the otherwise-idle ACT queue in parallel.
        nc.scalar.dma_start(out=w_sb["wkh"], in_=w_d["wkh"].ap())
        nc.gpsimd.dma_start(out=w_sb["wkl"], in_=w_d["wkl"].ap())
        dma_x(0, split=True)
        for n in ("wqh", "wql", "wvh", "wvl"):
            nc.gpsimd.dma_start(out=w_sb[n], in_=w_d[n].ap())
        dma_x(1)

        # --- single psum pool for the whole kernel (no phase handover):
        # big_ps [128,512] x4 banks + pv_ps [128,257] x4 banks = 8 banks
        psum = ctx.enter_context(
            tc.tile_pool(name="psum", bufs=4, space=bass.MemorySpace.PSUM)
        )

        # PE warm-up: tiny-FD matmuls — enough wall-clock to ramp the
        # p-state and cover the first DMAs, at minimal cycle cost
        jt = singles.tile([128, 512], BF16, tag="junk")
        nc.vector.memset(jt[:, 0:128], 0.0)
        jps = psum.tile([128, 512], F32, tag="big_ps", bufs=4, name="jps")
        for w in range(72):
            nc.tensor.matmul(jps[:, 0:64], jt[:, 0:128], jt[:, 0:64],
                             start=True, stop=True)
        nc.vector.tensor_copy(jt[:, 0:64], jps[:, 0:64])

        def proj_cols(ps, wname, hsl, xh_t, xl_t, col0, ncol):
            """3-term DoubleRow accumulation of one [128, ncol] output."""
            first = True
            for wn, xt in ((wname + "h", xh_t), (wname + "h", xl_t),
                           (wname + "l", xh_t)):
                for dp in range(DP):
                    nc.tensor.matmul(
                        ps,
                        w_sb[wn][:, 2 * dp:2 * dp + 2, hsl],
                        xt[:, dp, :, col0:col0 + ncol],
                        start=first,
                        stop=(wn == wname + "l" and dp == DP - 1),
                        perf_mode=DR,
                    )
                    first = False

        N_PRE = 8
        pre_ets = []
        for sc in range(SC):
            if sc + 2 < SC:
                dma_x(sc + 2)
            xh_t, xl_t = x_tiles.pop(sc)

            projs = ([("wk", kT)] if sc < KC else []) + [("wq", qT)]
            for wname, dst in projs:
                for h in range(2):
                    ps = psum.tile([128, 512], F32, tag="big_ps", bufs=4,
                                   name=f"ps{sc}{wname}{h}")
                    proj_cols(ps, wname, slice(h * HD, (h + 1) * HD),
                              xh_t, xl_t, 0, 512)
                    nc.vector.tensor_copy(dst[:, h, sc * 512:(sc + 1) * 512], ps)
            if sc < KC:
                for i in range(4):
                    vt = psum.tile([128, DVA], F32, tag="pv_ps", bufs=4,
                                   name=f"vps{sc}_{i}")
                    vps = vt[:, 0:DV]
                    first = True
                    for wn, xt in (("wvh", xh_t), ("wvh", xl_t), ("wvl", xh_t)):
                        for dp in range(DP):
                            nc.tensor.matmul(
                                vps,
                                xt[:, dp, :, i * 128:(i + 1) * 128],
                                w_sb[wn][:, 2 * dp:2 * dp + 2, :],
                                start=first,
                                stop=(wn == "wvl" and dp == DP - 1),
                                perf_mode=DR,
                            )
                            first = False
                    nc.vector.tensor_copy(v_aug[:, sc * 4 + i, 0:DV], vps)

            if 3 <= sc <= 6:
                # prefill the first attention key tiles' scores+exp during
                # the Q-proj tail (2 per chunk — psum rotation stays free):
                # ACT absorbs the exps, so PV starts instantly in attention
                for skt in (2 * (sc - 3), 2 * (sc - 3) + 1):
                    ps = psum.tile([128, 512], F32, tag="big_ps", bufs=4,
                                   name=f"pre_sps{skt}")
                    nc.tensor.matmul(ps, kT[:, 0, skt * 128:(skt + 1) * 128],
                                     qT[:, 0, 0:512])
                    et = e_pool.tile([128, 512], BF16, tag="et",
                                     name=f"pre_et{skt}")
                    nc.scalar.activation(
                        out=et, in_=ps,
                        func=mybir.ActivationFunctionType.Exp,
                        scale=SCALE / (WSC * WSC),
                    )
                    pre_ets.append(et)

        # --- attention: per (q-chunk, head), partial PV over key half ---
        for qc in range(QC):
            ot = o_pool.tile([128, 2, 4, DVA], F32, tag="ot", name=f"ot{qc}")
            for h in range(2):
                pv_ps = [
                    psum.tile([128, DVA], F32, tag="pv_ps", bufs=4, name=f"pv_ps{i}")
                    for i in range(4)
                ]
                for skt in range(SKT):
                    if qc == 0 and h == 0 and skt < N_PRE:
                        et = pre_ets[skt]
                    else:
                        sps = psum.tile([128, 512], F32, tag="big_ps", bufs=4,
                                        name=f"sps{skt}")
                        nc.tensor.matmul(
                            sps,
                            kT[:, h, skt * 128:(skt + 1) * 128],
                            qT[:, h, qc * 512:(qc + 1) * 512],
                        )
                        et = e_pool.tile([128, 512], BF16, tag="et",
                                         name=f"et{skt}")
                        nc.scalar.activation(
                            out=et, in_=sps,
                            func=mybir.ActivationFunctionType.Exp,
                            scale=SCALE / (WSC * WSC),
                        )
                    for i in range(4):
                        nc.tensor.matmul(
                            pv_ps[i],
                            et[:, i * 128:(i + 1) * 128],
                            v_aug[:, skt, :],
                            start=(skt == 0),
                            stop=(skt == SKT - 1),
                        )
                if qc == QC - 1 and h == 1:
                    # final tiles: copies split across DVE/GpSimd and drained
                    # by per-tile DMAs from the same engine to cut the tail
                    for i in range(4):
                        deng = (nc.sync, nc.gpsimd, nc.scalar, nc.sync)[i]
                        nc.vector.tensor_copy(ot[:, h, i, :], pv_ps[i])
                        deng.dma_start(out=out[:, qc, h, i, :],
                                       in_=ot[:, h, i, :])
                else:
                    for i in range(4):
                        nc.vector.tensor_copy(ot[:, h, i, :], pv_ps[i])
                    if qc == QC - 1:
                        nc.sync.dma_start(out=out[:, qc, h, :, :],
                                          in_=ot[:, h, :, :])
            if qc < QC - 1:
                nc.sync.dma_start(out=out[:, qc, :, :, :], in_=ot)

    nc.compile()
    return nc


def _lam(lambda_q1, lambda_q2, lambda_k1, lambda_k2):
    lam_init = 0.8 - 0.6 * math.exp(-0.3 * DEPTH)
    l1 = math.exp(float(np.sum(lambda_q1.astype(np.float64) * lambda_k1.astype(np.float64))))
    l2 = math.exp(float(np.sum(lambda_q2.astype(np.float64) * lambda_k2.astype(np.float64))))
    return l1 + l2 + lam_init


def _split_x(xT):
    """xT [D, S] f32 -> (hi, lo) e4m3 in [128, SC, DP, 2, 512] layout."""
    xh = xT.astype(E4)
    xl = (xT - xh.astype(np.float32)).astype(E4)
    out = []
    for a in (xh, xl):
        a = a.reshape(DP, 2, 128, SC, 512).transpose(2, 3, 0, 1, 4)
        out.append(np.ascontiguousarray(a))
    return out


def _split_w(W):
    """W [D, DV] f32 -> (hi, lo) e4m3 in [128, DT_P, DV] layout."""
    Ws = W.astype(np.float32) * WSC
    wh = Ws.astype(E4)
    wl = (Ws - wh.astype(np.float32)).astype(E4)
    out = []
    for a in (wh, wl):
        a = a.reshape(DT_P, 128, DV).transpose(1, 0, 2)
        out.append(np.ascontiguousarray(a))
    return out


def kernel(x, WQ, WK, WV, lambda_q1, lambda_q2, lambda_k1, lambda_k2):
    if "nc" not in _cache:
        _cache["nc"] = build_nc()
    nc = _cache["nc"]

    wqh, wql = _split_w(WQ)
    wkh, wkl = _split_w(WK)
    wvh, wvl = _split_w(WV)
    lam = _lam(lambda_q1, lambda_q2, lambda_k1, lambda_k2)

    in_maps = []
    for c in range(N_CORES):
        b, kh = c // 2, c % 2
        xb = x[b] if kh == 0 else np.concatenate([x[b, SK:], x[b, :SK]], axis=0)
        xT = np.ascontiguousarray(xb.T, dtype=np.float32)
        xh, xl = _split_x(xT)
        in_maps.append({
            "xh": xh, "xl": xl,
            "wqh": wqh, "wql": wql, "wkh": wkh, "wkl": wkl,
            "wvh": wvh, "wvl": wvl,
        })

    kres = None
    for attempt in range(3):
        try:
            kres = run_bass_kernel_spmd(nc, in_maps, list(range(N_CORES)))
            break
        except (ModuleNotFoundError, ImportError):
            # BASS_TRACE requested but this axon build has no NTFF hook
            os.environ["BASS_NEVER_TRACE"] = "1"
        except Exception:
            if attempt == 2:
                raise
            time.sleep(5)
    if kres is None:
        kres = run_bass_kernel_spmd(nc, in_maps, list(range(N_CORES)))
    _cache["last_results"] = kres
    res = kres.results

    out = np.empty((B, S, DV), np.float32)
    for b in range(B):
        # out tensor is [128, QC, 2, 4, DVA]: query index = (qc*4 + sub)*128 + p
        def decode(a):
            # [p, qc, h, sub, dva] -> [h, qc, sub, p, dva] -> [h, S, DVA]
            return a.transpose(2, 1, 3, 0, 4).reshape(2, S, DVA)
        a0 = decode(res[2 * b]["out"])
        a1 = decode(res[2 * b + 1]["out"])
        a1 = np.concatenate([a1[:, SK:], a1[:, :SK]], axis=1)  # un-rotate
        acc = a0.astype(np.float64) + a1.astype(np.float64)
        o1 = acc[0, :, :DV] / acc[0, :, DV:DVA]
        o2 = acc[1, :, :DV] / acc[1, :, DV:DVA]
        out[b] = (o1 - lam * o2).astype(np.float32)
    return out


# revision 40
# speedup vs baseline: 1.2265x; 1.0013x over previous
"""Differential attention kernel for 8 Trainium2 NeuronCores — v4.

Sharding: (batch, key-half) per core. Each core projects K/V for its
2048-key half only (dedup vs v3's query-split, which duplicated K/V
across the pair), projects Q for all 4096 queries, and computes partial
PV + row-sum accumulators for both heads over its key half. The host
sums the two partials per batch and applies the differential-softmax
normalization (o1/r1 - lam*o2/r2) in numpy.

Projections run as fp8-e4m3 DoubleRow matmuls (cost 0.5 cycles/row,
256-wide contraction) with a 3-term hi/lo residual split:
  x @ W  ~=  xh@Wh + xl@Wh + xh@Wl      (lo*lo term dropped, ~eps^2)
which keeps bf16-level accuracy (measured 3.9e-3 vs 4.5e-3 all-bf16)
at 0.75x the PE cycles. Scores and PV stay bf16 (fp8 cannot hold the
exp() dynamic range; measured catastrophic).

Host preps x/W hi+lo splits and interleaved DRAM layouts so each weight
is one DMA and each x chunk is two (hi/lo).
"""

import math
import os
import time
from contextlib import ExitStack

import ml_dtypes
import numpy as np

import concourse.bass as bass
from concourse import bacc
import concourse.mybir as mybir
import concourse.tile as tile
from concourse.bass_utils import run_bass_kernel_spmd

B, S, D = 4, 4096, 2048
HD = 128
DV = 256
DVA = DV + 1      # + ones column for row sums
SK = S // 2       # keys per core (key-half)
N_CORES = 8
DEPTH = 12
SCALE = HD ** -0.5
WSC = 64.0        # host-side weight scale before fp8 split

DT_P = D // 128   # 16 d-tiles
DP = DT_P // 2    # 8 d-pairs (DoubleRow contraction = 256)
SC = S // 512     # 8 s-chunks (queries)
KC = SK // 512    # 4 s-chunks that are also key chunks
SKT = SK // 128   # 16 key tiles
QC = S // 512     # 8 attention q-chunks
QT = S // 128     # 32 q tiles

BF16 = mybir.dt.bfloat16
F32 = mybir.dt.float32
FP8 = mybir.dt.float8e4
DR = mybir.MatmulPerfMode.DoubleRow
E4 = ml_dtypes.float8_e4m3

INPUT_NAMES = ("xh", "xl", "wqh", "wql", "wkh", "wkl", "wvh", "wvl")

_cache = {}


def build_nc():
    nc = bacc.Bacc("TRN2", target_bir_lowering=False, debug=False)

    # x split halves, host-arranged as [p, chunk, dpair, j, col] so one
    # chunk slice is a single contiguous 8KB descriptor per partition.
    xh_d = nc.declare_dram_parameter("xh", [128, SC, DP, 2, 512], FP8, isOutput=False)
    xl_d = nc.declare_dram_parameter("xl", [128, SC, DP, 2, 512], FP8, isOutput=False)
    w_d = {}
    for n in ("wqh", "wql", "wkh", "wkl", "wvh", "wvl"):
        # host-arranged [p, dtile, col]
        w_d[n] = nc.declare_dram_parameter(n, [128, DT_P, DV], FP8, isOutput=False)
    # out[p, qc, h, sub, dva]: one contiguous-per-partition DMA per q-chunk
    out_d = nc.declare_dram_parameter("out", [128, QC, 2, 4, DVA], F32, isOutput=True)

    out = out_d.ap()

    with tile.TileContext(nc) as tc, ExitStack() as ctx:
        singles = ctx.enter_context(tc.tile_pool(name="singles", bufs=1))
        x_pool = ctx.enter_context(tc.tile_pool(name="x", bufs=6))
        e_pool = ctx.enter_context(tc.tile_pool(name="e", bufs=12))
        o_pool = ctx.enter_context(tc.tile_pool(name="o", bufs=4))

        # --- resident SBUF tensors --------------------------------------
        w_sb = {n: singles.tile([128, DT_P, DV], FP8, tag=f"w_{n}", name=f"w_{n}")
                for n in w_d}

        x_tiles = {}

        def dma_x(sc, split=False):
            # DMA issue is split across the otherwise-idle engines: the cost
            # model charges the transfer to the issuing engine's timeline, so
            # xh rides SP while xl rides Pool (concurrent streams).
            xh_t = x_pool.tile([128, DP, 2, 512], FP8, tag="xt", name=f"xh{sc}")
            xl_t = x_pool.tile([128, DP, 2, 512], FP8, tag="xt", name=f"xl{sc}")
            if split:
                # chunk 0: per-dpair pieces so the first matmuls start early
                for dp in range(DP):
                    nc.sync.dma_start(out=xh_t[:, dp, :, :],
                                      in_=xh_d.ap()[:, sc, dp, :, :])
                    nc.gpsimd.dma_start(out=xl_t[:, dp, :, :],
                                        in_=xl_d.ap()[:, sc, dp, :, :])
            else:
                nc.sync.dma_start(out=xh_t, in_=xh_d.ap()[:, sc, :, :, :])
                nc.gpsimd.dma_start(out=xl_t, in_=xl_d.ap()[:, sc, :, :, :])
            x_tiles[sc] = (xh_t, xl_t)

        kT = singles.tile([128, 2, SK], BF16, tag="kT")       # [dh, head, sk]
        qT = singles.tile([128, 2, S], BF16, tag="qT")        # [dh, head, sq]
        v_aug = singles.tile([128, SKT, DVA], BF16, tag="v")  # [s_row, s_tile, dv+1]

        # junk-warmup operand memset comes first so PE can start sooner;
        # ones column carries the row sums; 64 cancels the weight scale
        jt = singles.tile([128, 512], BF16, tag="junk")
        nc.vector.memset(jt[:, 0:128], 0.0)
        nc.vector.memset(v_aug[:, :, DV:DVA], WSC)

        # DMA issue order: chunk-0 xh pieces stream on SP from t=0 while the
        # first weight rides the otherwise-idle ACT queue in parallel.
        nc.scalar.dma_start(out=w_sb["wkh"], in_=w_d["wkh"].ap())
        nc.gpsimd.dma_start(out=w_sb["wkl"], in_=w_d["wkl"].ap())
        dma_x(0, split=True)
        for n in ("wqh", "wql", "wvh", "wvl"):
            nc.gpsimd.dma_start(out=w_sb[n], in_=w_d[n].ap())
        dma_x(1)

        # --- single psum pool for the whole kernel (no phase handover):
        # big_ps [128,512] x4 banks + pv_ps [128,257] x4 banks = 8 banks
        psum = ctx.enter_context(
            tc.tile_pool(name="psum", bufs=4, space=bass.MemorySpace.PSUM)
        )

        # PE warm-up: tiny-FD matmuls — enough wall-clock to ramp the
        # p-state and cover the first DMAs, at minimal cycle cost
        jps = psum.tile([128, 512], F32, tag="big_ps", bufs=4, name="jps")
        for w in range(72):
            nc.tensor.matmul(jps[:, 0:64], jt[:, 0:128], jt[:, 0:64],
                             start=True, stop=True)
        nc.vector.tensor_copy(jt[:, 0:64], jps[:, 0:64])

        def proj_cols(ps, wname, hsl, xh_t, xl_t, col0, ncol):
            """3-term DoubleRow accumulation of one [128, ncol] output."""
            first = True
            for wn, xt in ((wname + "h", xh_t), (wname + "h", xl_t),
                           (wname + "l", xh_t)):
                for dp in range(DP):
                    nc.tensor.matmul(
                        ps,
                        w_sb[wn][:, 2 * dp:2 * dp + 2, hsl],
                        xt[:, dp, :, col0:col0 + ncol],
                        start=first,
                        stop=(wn == wname + "l" and dp == DP - 1),
                        perf_mode=DR,
                    )
                    first = False

        N_PRE = 8
        pre_ets = []
        for sc in range(SC):
            if sc + 2 < SC:
                dma_x(sc + 2)
            xh_t, xl_t = x_tiles.pop(sc)

            projs = ([("wk", kT)] if sc < KC else []) + [("wq", qT)]
            for wname, dst in projs:
                for h in range(2):
                    ps = psum.tile([128, 512], F32, tag="big_ps", bufs=4,
                                   name=f"ps{sc}{wname}{h}")
                    proj_cols(ps, wname, slice(h * HD, (h + 1) * HD),
                              xh_t, xl_t, 0, 512)
                    nc.vector.tensor_copy(dst[:, h, sc * 512:(sc + 1) * 512], ps)
            if sc < KC:
                for i in range(4):
                    vt = psum.tile([128, DVA], F32, tag="pv_ps", bufs=4,
                                   name=f"vps{sc}_{i}")
                    vps = vt[:, 0:DV]
                    first = True
                    for wn, xt in (("wvh", xh_t), ("wvh", xl_t), ("wvl", xh_t)):
                        for dp in range(DP):
                            nc.tensor.matmul(
                                vps,
                                xt[:, dp, :, i * 128:(i + 1) * 128],
                                w_sb[wn][:, 2 * dp:2 * dp + 2, :],
                                start=first,
                                stop=(wn == "wvl" and dp == DP - 1),
                                perf_mode=DR,
                            )
                            first = False
                    nc.vector.tensor_copy(v_aug[:, sc * 4 + i, 0:DV], vps)

            if 3 <= sc <= 6:
                # prefill the first attention key tiles' scores+exp during
                # the Q-proj tail (2 per chunk — psum rotation stays free):
                # ACT absorbs the exps, so PV starts instantly in attention
                for skt in (2 * (sc - 3), 2 * (sc - 3) + 1):
                    ps = psum.tile([128, 512], F32, tag="big_ps", bufs=4,
                                   name=f"pre_sps{skt}")
                    nc.tensor.matmul(ps, kT[:, 0, skt * 128:(skt + 1) * 128],
                                     qT[:, 0, 0:512])
                    et = e_pool.tile([128, 512], BF16, tag="et",
                                     name=f"pre_et{skt}")
                    nc.scalar.activation(
                        out=et, in_=ps,
                        func=mybir.ActivationFunctionType.Exp,
                        scale=SCALE / (WSC * WSC),
                    )
                    pre_ets.append(et)

        # --- attention: per (q-chunk, head), partial PV over key half ---
        for qc in range(QC):
            ot = o_pool.tile([128, 2, 4, DVA], F32, tag="ot", name=f"ot{qc}")
            for h in range(2):
                pv_ps = [
                    psum.tile([128, DVA], F32, tag="pv_ps", bufs=4, name=f"pv_ps{i}")
                    for i in range(4)
                ]
                for skt in range(SKT):
                    if qc == 0 and h == 0 and skt < N_PRE:
                        et = pre_ets[skt]
                    else:
                        sps = psum.tile([128, 512], F32, tag="big_ps", bufs=4,
                                        name=f"sps{skt}")
                        nc.tensor.matmul(
                            sps,
                            kT[:, h, skt * 128:(skt + 1) * 128],
                            qT[:, h, qc * 512:(qc + 1) * 512],
                        )
                        et = e_pool.tile([128, 512], BF16, tag="et",
                                         name=f"et{skt}")
                        nc.scalar.activation(
                            out=et, in_=sps,
                            func=mybir.ActivationFunctionType.Exp,
                            scale=SCALE / (WSC * WSC),
                        )
                    for i in range(4):
                        nc.tensor.matmul(
                            pv_ps[i],
                            et[:, i * 128:(i + 1) * 128],
                            v_aug[:, skt, :],
                            start=(skt == 0),
                            stop=(skt == SKT - 1),
                        )
                if qc == QC - 1 and h == 1:
                    # final tiles: copies split across DVE/GpSimd and drained
                    # by per-tile DMAs from the same engine to cut the tail
                    for i in range(4):
                        deng = (nc.sync, nc.gpsimd, nc.sync, nc.gpsimd)[i]
                        if i % 2 == 0:
                            nc.vector.tensor_copy(ot[:, h, i, :], pv_ps[i])
                        else:
                            # ACT copy shares the exp table set — no reload
                            nc.scalar.activation(
                                out=ot[:, h, i, :], in_=pv_ps[i],
                                func=mybir.ActivationFunctionType.Copy,
                            )
                        deng.dma_start(out=out[:, qc, h, i, :],
                                       in_=ot[:, h, i, :])
                else:
                    for i in range(4):
                        nc.vector.tensor_copy(ot[:, h, i, :], pv_ps[i])
                    if qc == QC - 1:
                        nc.sync.dma_start(out=out[:, qc, h, :, :],
                                          in_=ot[:, h, :, :])
            if qc < QC - 1:
                nc.sync.dma_start(out=out[:, qc, :, :, :], in_=ot)

    nc.compile()
    return nc


def _lam(lambda_q1, lambda_q2, lambda_k1, lambda_k2):
    lam_init = 0.8 - 0.6 * math.exp(-0.3 * DEPTH)
    l1 = math.exp(float(np.sum(lambda_q1.astype(np.float64) * lambda_k1.astype(np.float64))))
    l2 = math.exp(float(np.sum(lambda_q2.astype(np.float64) * lambda_k2.astype(np.float64))))
    return l1 + l2 + lam_init


def _split_x(xT):
    """xT [D, S] f32 -> (hi, lo) e4m3 in [128, SC, DP, 2, 512] layout."""
    xh = xT.astype(E4)
    xl = (xT - xh.astype(np.float32)).astype(E4)
    out = []
    for a in (xh, xl):
        a = a.reshape(DP, 2, 128, SC, 512).transpose(2, 3, 0, 1, 4)
        out.append(np.ascontiguousarray(a))
    return out


def _split_w(W):
    """W [D, DV] f32 -> (hi, lo) e4m3 in [128, DT_P, DV] layout."""
    Ws = W.astype(np.float32) * WSC
    wh = Ws.astype(E4)
    wl = (Ws - wh.astype(np.float32)).astype(E4)
    out = []
    for a in (wh, wl):
        a = a.reshape(DT_P, 128, DV).transpose(1, 0, 2)
        out.append(np.ascontiguousarray(a))
    return out


def kernel(x, WQ, WK, WV, lambda_q1, lambda_q2, lambda_k1, lambda_k2):
    if "nc" not in _cache:
        _cache["nc"] = build_nc()
    nc = _cache["nc"]

    wqh, wql = _split_w(WQ)
    wkh, wkl = _split_w(WK)
    wvh, wvl = _split_w(WV)
    lam = _lam(lambda_q1, lambda_q2, lambda_k1, lambda_k2)

    in_maps = []
    for c in range(N_CORES):
        b, kh = c // 2, c % 2
        xb = x[b] if kh == 0 else np.concatenate([x[b, SK:], x[b, :SK]], axis=0)
        xT = np.ascontiguousarray(xb.T, dtype=np.float32)
        xh, xl = _split_x(xT)
        in_maps.append({
            "xh": xh, "xl": xl,
            "wqh": wqh, "wql": wql, "wkh": wkh, "wkl": wkl,
            "wvh": wvh, "wvl": wvl,
        })

    kres = None
    for attempt in range(3):
        try:
            kres = run_bass_kernel_spmd(nc, in_maps, list(range(N_CORES)))
            break
        except (ModuleNotFoundError, ImportError):
            # BASS_TRACE requested but this axon build has no NTFF hook
            os.environ["BASS_NEVER_TRACE"] = "1"
        except Exception:
            if attempt == 2:
                raise
            time.sleep(5)
    if kres is None:
        kres = run_bass_kernel_spmd(nc, in_maps, list(range(N_CORES)))
    _cache["last_results"] = kres
    res = kres.results

    out = np.empty((B, S, DV), np.float32)
    for b in range(B):
        # out tensor is [128, QC, 2, 4, DVA]: query index = (qc*4 + sub)*128 + p
        def decode(a):
            # [p, qc, h, sub, dva] -> [h, qc, sub, p, dva] -> [h, S, DVA]
            return a.transpose(2, 1, 3, 0, 4).reshape(2, S, DVA)
        a0 = decode(res[2 * b]["out"])
        a1 = decode(res[2 * b + 1]["out"])
        a1 = np.concatenate([a1[:, SK:], a1[:, :SK]], axis=1)  # un-rotate
        acc = a0.astype(np.float64) + a1.astype(np.float64)
        o1 = acc[0, :, :DV] / acc[0, :, DV:DVA]
        o2 = acc[1, :, :DV] / acc[1, :, DV:DVA]
        out[b] = (o1 - lam * o2).astype(np.float32)
    return out
